# revision 1
# baseline (speedup 1.0000x reference)
"""Trainium2 Bass kernel for nn_DenoiseEncoderTransformer.

Model (reference.py): B=4, T=2048, D=128, V=64, L=12, HM=512 encoder
transformer; quadratic token embeddings -(c-x)^2/2 padded to D, plus
pos_emb; 12 pre-LN layers (single-head full attention D=128, MLP 128->512
->128 with exact gelu); scalar readout head.

Sharding: 8 cores = 4 batches x 2 sequence halves. Each core owns 1024
query rows of one batch element. Per layer, K^T/V^T halves are exchanged
between the two cores of a batch via a 2-rank AllGather; everything else
is row-local. A 4-core fallback (one core per batch, no collectives) is
also supported.

Compute dtype: float32r (fp32 bits, PE rounds; 1 cycle/row for moving
free dim >= 256, ~1.7e-4 matmul rel err). LayerNorm gains/biases are
folded into the projection weights host-side; matmuls run feature-major
(contraction dim on partitions) with PE transposes where layouts flip.

NOTE: sub-512B DMA descriptors are hazardous on this stack (two small
DMAs corrupt a following DMA) — all small per-partition params are
consolidated into one [128, 128] f32 "smalls" tensor loaded by a single
512B-per-partition DMA.
"""

import os
from contextlib import ExitStack

import ml_dtypes
import numpy as np

import concourse.bass as bass
import concourse.tile as tile
from concourse import bacc, mybir
from concourse.bass_utils import run_bass_kernel_spmd
from concourse.masks import make_identity

# model constants (hardcoded per problem statement)
B, T, D, V, L, HM = 4, 2048, 128, 64, 12, 512
EPS = 1e-5
SCALE = float(np.sqrt(D))

F32 = mybir.dt.float32
F32R = mybir.dt.float32r
BF16 = mybir.dt.bfloat16
I32 = mybir.dt.int32

MODE = os.environ.get("DET_MODE", "m8")  # "m8": 8 cores, "m4": 4 cores

# column layout of the consolidated "smalls" [128, 128] tensor
_C_CQKV = 0            # 3 cols per layer: cq, ck, cv        (36)
_C_C1 = 3 * L          # 4 cols per layer: c1 per hm-tile    (48)
_C_C2 = 7 * L          # 1 col per layer: c2                 (12)
_C_WRO = 8 * L         # 1 col: Wro^T                        (1)
_C_BRO = 8 * L + 1     # 1 col: bro broadcast                (1)


def _build(mode: str):
    """Build + compile the SPMD program. Returns (nc, n_cores)."""
    n_layers = int(os.environ.get("DET_NLAYERS", str(L)))
    dump_hc = os.environ.get("DET_DUMP", "0") == "1"
    n_cores = 8 if mode == "m8" else 4
    TL = T // 2 if mode == "m8" else T      # local residual rows per core
    NT = TL // 128                           # local 128-row chunks
    NQ = 1 if mode == "m8" else 2            # q-chunks of 1024 rows
    TK = T                                   # full key length
    NKT = TK // 128                          # key tiles

    nc = bacc.Bacc("TRN2", target_bir_lowering=False, debug=False,
                   num_devices=n_cores)

    # ---- DRAM I/O ----
    xin = nc.dram_tensor("xin", [1, TL + 128], I32, kind="ExternalInput")
    pose = nc.dram_tensor("pose", [TL, D], F32, kind="ExternalInput")
    aconst = nc.dram_tensor("aconst", [3, D], F32R, kind="ExternalInput")
    wqkT = nc.dram_tensor("wqkT", [L, 2, D, D], BF16, kind="ExternalInput")
    wvT = nc.dram_tensor("wvT", [L, D, D], BF16, kind="ExternalInput")
    w1T = nc.dram_tensor("w1T", [L, D, HM], BF16, kind="ExternalInput")
    w2T = nc.dram_tensor("w2T", [L, HM, D], BF16, kind="ExternalInput")
    smalls = nc.dram_tensor("smalls", [D, 128], F32, kind="ExternalInput")
    out = nc.dram_tensor("out", [1, TL], F32, kind="ExternalOutput")
    if dump_hc:
        dbg = nc.dram_tensor("dbg", [128, NT, 128], F32,
                             kind="ExternalOutput")

    with tile.TileContext(nc) as tc, ExitStack() as ctx:
        # ---- pools ----
        singles = ctx.enter_context(tc.tile_pool(name="singles", bufs=1))
        persist = ctx.enter_context(tc.tile_pool(name="persist", bufs=1))
        params = ctx.enter_context(tc.tile_pool(name="params", bufs=2))
        work = ctx.enter_context(tc.tile_pool(name="work", bufs=1))
        et_pool = ctx.enter_context(tc.tile_pool(name="et", bufs=4))
        small = ctx.enter_context(tc.tile_pool(name="small", bufs=4))
        ps_gen = ctx.enter_context(
            tc.tile_pool(name="ps_gen", bufs=4, space="PSUM"))
        ps_acc = ctx.enter_context(
            tc.tile_pool(name="ps_acc", bufs=4, space="PSUM"))
        if mode == "m8":
            dram = ctx.enter_context(
                tc.tile_pool(name="dram", bufs=2, space="DRAM"))

        # ---- one-time setup ----
        ident_f = singles.tile([128, 128], F32)
        make_identity(nc, ident_f)
        ident = singles.tile([128, 128], F32R)
        nc.scalar.copy(ident, ident_f)
        ident_b = singles.tile([128, 128], BF16)
        nc.scalar.copy(ident_b, ident_f)
        ones_f = singles.tile([128, 128], F32)
        nc.gpsimd.memset(ones_f, 1.0)
        ones_t = singles.tile([128, 128], BF16)
        nc.scalar.copy(ones_t, ones_f)
        eps_t = singles.tile([128, 1], F32)
        nc.vector.memset(eps_t, EPS)

        sm = singles.tile([D, 128], F32)
        nc.sync.dma_start(sm, smalls[:, :])
        acon_s = singles.tile([3, D], F32R)
        nc.sync.dma_start(acon_s, aconst[:, :])

        # ---- embedding: Hc[t, d] = -(c - x)^2/2 (padded) + pos_emb ----
        x_i = singles.tile([1, TL + 128], I32)
        nc.sync.dma_start(x_i, xin[:, :])
        x_f = singles.tile([1, TL], F32)
        nc.vector.tensor_copy(x_f, x_i[:, 0:TL])
        x_sq = singles.tile([1, TL], F32)
        nc.scalar.square(x_sq, x_f)
        xf_f = singles.tile([3, TL], F32)
        nc.vector.memset(xf_f, 1.0)
        nc.sync.dma_start(xf_f[1:2, :], x_f)
        nc.sync.dma_start(xf_f[2:3, :], x_sq)
        xf = singles.tile([3, TL], F32R)
        nc.scalar.copy(xf, xf_f)

        Hc = persist.tile([128, NT, 128], F32)  # residual, token-major
        pos_s = work.tile([128, NT, 128], F32)
        nc.sync.dma_start(pos_s, pose.rearrange("(n p) d -> p n d", p=128))
        for i in range(NT):
            pse = ps_gen.tile([128, 128], F32, tag="psg")
            nc.tensor.matmul(pse, xf[:, bass.ts(i, 128)], acon_s,
                             start=True, stop=True)
            nc.vector.tensor_add(Hc[:, i, :], pse, pos_s[:, i, :])

        def layernorm_T(src_chunks, dst, tagp):
            """LN(src) per 128-row chunk -> transpose -> dst [d, TL] bf16."""
            mvall = small.tile([128, NT, 2], F32, tag=f"mv{tagp}")
            for i in range(NT):
                stats = small.tile([128, 6], F32, tag=f"st{tagp}")
                nc.vector.bn_stats(stats, src_chunks[:, i, :])
                nc.vector.bn_aggr(mvall[:, i, :], stats)
            rstd_all = small.tile([128, NT], F32, tag=f"rs{tagp}")
            nc.scalar.activation(rstd_all, mvall[:, :, 1],
                                 mybir.ActivationFunctionType.Sqrt,
                                 bias=eps_t, scale=1.0)
            nc.vector.reciprocal(rstd_all, rstd_all)
            for i in range(NT):
                z = small.tile([128, 128], BF16, tag=f"z{tagp}")
                nc.vector.tensor_scalar(
                    out=z, in0=src_chunks[:, i, :],
                    scalar1=mvall[:, i, 0:1], scalar2=rstd_all[:, i:i + 1],
                    op0=mybir.AluOpType.subtract, op1=mybir.AluOpType.mult)
                pst = ps_gen.tile([128, 128], BF16, tag="psg")
                nc.tensor.transpose(pst, z, ident_b)
                nc.vector.tensor_copy(dst[:, bass.ts(i, 128)], pst)

        for l in range(n_layers):
            # ---- LN1 + transpose ----
            z1T = work.tile([128, TL], BF16, tag="zT")
            layernorm_T(Hc, z1T, "a")

            # ---- QKV projections (feature-major) ----
            wqk_s = params.tile([128, 2, 128], BF16, tag="wqk")
            nc.sync.dma_start(wqk_s, wqkT[l].rearrange("a p d -> p a d"))
            wv_s = params.tile([128, 128], BF16, tag="wv")
            nc.sync.dma_start(wv_s, wvT[l])

            QT = work.tile([128, TL], BF16, tag="QT")
            KTl = work.tile([128, TL], BF16, tag="KTl")
            VTl = work.tile([128, TL], BF16, tag="VTl")

            def proj(w_ap, c_i, dst, f32_dst=None):
                cq_ap = sm[:, _C_CQKV + 3 * l + c_i:_C_CQKV + 3 * l + c_i + 1]
                for j in range(TL // 512):
                    psq = ps_gen.tile([128, 512], F32, tag="psg")
                    nc.tensor.matmul(psq, w_ap, z1T[:, bass.ts(j, 512)],
                                     start=True, stop=True)
                    nc.scalar.activation(
                        dst[:, bass.ts(j, 512)], psq,
                        mybir.ActivationFunctionType.Identity,
                        bias=cq_ap, scale=1.0)
                    if f32_dst is not None:
                        nc.vector.tensor_scalar_add(
                            f32_dst[:, bass.ts(j, 512)], psq, cq_ap)

            proj(wqk_s[:, 1, :], 1, KTl)
            proj(wv_s[:, :], 2, VTl)
            if mode == "m8":
                kv_in = dram.tile([2, 128, TL], BF16, tag="kvi")
                nc.sync.dma_start(kv_in[0], KTl)
                nc.sync.dma_start(kv_in[1], VTl)
                kv_out = dram.tile([2, 2, 128, TL], BF16, tag="kvo")
                nc.gpsimd.collective_compute(
                    "AllGather", mybir.AluOpType.bypass,
                    ins=[kv_in[:].opt()], outs=[kv_out[:].opt()],
                    replica_groups=[[0, 1], [2, 3], [4, 5], [6, 7]],
                )
            proj(wqk_s[:, 0, :], 0, QT)

            # local V -> token-major tiles (keys processed local-half first)
            NLOC = TL // 128
            Vtok = work.tile([128, NKT, 128], BF16, tag="Vtok")
            for k in range(NLOC):
                psv = ps_gen.tile([128, 128], BF16, tag="psg")
                nc.tensor.transpose(psv, VTl[:, bass.ts(k, 128)], ident_b)
                nc.vector.tensor_copy(Vtok[:, k, :], psv)

            # ---- attention, per q-chunk of 1024 ----
            def attn_tiles(qc, krange, accs):
                qs = qc * 1024
                for k in krange:
                    first, last = (k == 0), (k == NKT - 1)
                    if k < NLOC:
                        kt = KTl[:, bass.ts(k, 128)]
                    else:
                        kt = KTr[:, bass.ts(k - NLOC, 128)]
                    for (half, u_ps, r_ps) in ((0, accs[0], accs[2]),
                                               (1, accs[1], accs[3])):
                        pss = ps_gen.tile([128, 512], F32, tag="psg")
                        nc.tensor.matmul(
                            pss, kt, QT[:, bass.ds(qs + half * 512, 512)],
                            start=True, stop=True)
                        et = et_pool.tile([128, 512], BF16, tag="et")
                        nc.scalar.activation(
                            et, pss, mybir.ActivationFunctionType.Exp,
                            scale=1.0 / SCALE)
                        nc.tensor.matmul(u_ps, Vtok[:, k, :], et,
                                         start=first, stop=last)
                        nc.tensor.matmul(r_ps, ones_t, et,
                                         start=first, stop=last)

            accs_all = []
            for qc in range(NQ):
                accs = [ps_acc.tile([128, 512], F32, tag="psa",
                                    name=f"acc{qc}_{i}")
                        for i in range(4)]
                accs_all.append(accs)
                attn_tiles(qc, range(NLOC), accs)

            if mode == "m8":
                # partner half = (g0 + g1) - local, exact via f32 intermediate
                gK = work.tile([128, 2, TL], BF16, tag="gK")
                nc.sync.dma_start(gK[:], kv_out[:, 0].rearrange("r p t -> p r t"))
                gV = work.tile([128, 2, TL], BF16, tag="gV")
                nc.sync.dma_start(gV[:], kv_out[:, 1].rearrange("r p t -> p r t"))
                KVr = work.tile([128, 2, TL], BF16, tag="KVr")
                sK = work.tile([128, TL], F32, tag="sK")
                nc.vector.tensor_add(sK, gK[:, 0, :], gK[:, 1, :])
                nc.vector.tensor_tensor(
                    out=KVr[:, 0, :], in0=sK, in1=KTl,
                    op=mybir.AluOpType.subtract)
                sV = work.tile([128, TL], F32, tag="sV")
                nc.gpsimd.tensor_add(sV, gV[:, 0, :], gV[:, 1, :])
                nc.gpsimd.tensor_tensor(
                    out=KVr[:, 1, :], in0=sV, in1=VTl,
                    op=mybir.AluOpType.subtract)
                KTr = KVr[:, 0, :]
                for k in range(NLOC, NKT):
                    psv = ps_gen.tile([128, 128], BF16, tag="psg")
                    nc.tensor.transpose(
                        psv, KVr[:, 1, bass.ts(k - NLOC, 128)], ident_b)
                    nc.vector.tensor_copy(Vtok[:, k, :], psv)
                for qc in range(NQ):
                    attn_tiles(qc, range(NLOC, NKT), accs_all[qc])

            for qc in range(NQ):
                (uL, uR, rL, rR) = accs_all[qc]
                # normalize: UT / r  (r broadcast across partitions already)
                UTn = work.tile([128, 1024], BF16, tag="UTn")
                for (u_ps, r_ps, off) in ((uL, rL, 0), (uR, rR, 512)):
                    rrec = work.tile([128, 512], F32, tag="rrec")
                    nc.vector.reciprocal_approx_fast(rrec, r_ps)
                    nc.vector.tensor_mul(
                        UTn[:, bass.ds(off, 512)], u_ps, rrec)
                # transpose back + residual add
                for i in range(8):
                    psu = ps_gen.tile([128, 128], BF16, tag="psg")
                    nc.tensor.transpose(psu, UTn[:, bass.ts(i, 128)], ident_b)
                    ic = qc * 8 + i
                    nc.vector.tensor_add(Hc[:, ic, :], psu, Hc[:, ic, :])

            # ---- LN2 + transpose ----
            z2T = work.tile([128, TL], BF16, tag="zT")
            layernorm_T(Hc, z2T, "b")

            # ---- MLP ----
            w1_s = params.tile([128, HM], BF16, tag="w1")
            nc.sync.dma_start(w1_s, w1T[l])
            w2_s = params.tile([128, HM // 128, 128], BF16, tag="w2")
            nc.sync.dma_start(
                w2_s, w2T[l].rearrange("(m p) d -> p m d", p=128))

            OT = work.tile([128, TL], BF16, tag="OT")
            for j in range(TL // 512):
                gts = []
                for m in range(HM // 128):
                    psa = ps_gen.tile([128, 512], F32, tag="psg")
                    nc.tensor.matmul(psa, w1_s[:, bass.ts(m, 128)],
                                     z2T[:, bass.ts(j, 512)],
                                     start=True, stop=True)
                    gt = et_pool.tile([128, 512], BF16, tag="et")
                    c1_ap = sm[:, _C_C1 + 4 * l + m:_C_C1 + 4 * l + m + 1]
                    nc.scalar.activation(
                        gt, psa, mybir.ActivationFunctionType.Gelu,
                        bias=c1_ap, scale=1.0)
                    gts.append(gt)
                pso = ps_acc.tile([128, 512], F32, tag="psa")
                for m in range(HM // 128):
                    nc.tensor.matmul(pso, w2_s[:, m, :], gts[m],
                                     start=(m == 0),
                                     stop=(m == HM // 128 - 1))
                nc.vector.tensor_scalar_add(
                    OT[:, bass.ts(j, 512)], pso,
                    sm[:, _C_C2 + l:_C_C2 + l + 1])
            for i in range(NT):
                pst = ps_gen.tile([128, 128], BF16, tag="psg")
                nc.tensor.transpose(pst, OT[:, bass.ts(i, 128)], ident_b)
                nc.vector.tensor_add(Hc[:, i, :], pst, Hc[:, i, :])

        if dump_hc:
            nc.sync.dma_start(dbg[:, :, :], Hc)

        # ---- head: pred = Hc @ Wro^T + bro ----
        wro_s = singles.tile([128, 1], F32R)
        nc.scalar.copy(wro_s, sm[:, _C_WRO:_C_WRO + 1])
        HcT = work.tile([128, TL], F32R, tag="hct")
        for i in range(NT):
            hr = small.tile([128, 128], F32R, tag="hr")
            nc.vector.tensor_copy(hr, Hc[:, i, :])
            psh = ps_gen.tile([128, 128], F32R, tag="psg")
            nc.tensor.transpose(psh, hr, ident)
            nc.vector.tensor_copy(HcT[:, bass.ts(i, 128)], psh)
        pred = work.tile([1, TL], F32, tag="pred")
        for j in range(TL // 512):
            psp = ps_gen.tile([1, 512], F32, tag="psg")
            nc.tensor.matmul(psp, wro_s, HcT[:, bass.ts(j, 512)],
                             start=True, stop=True)
            nc.scalar.activation(pred[:, bass.ts(j, 512)], psp,
                                 mybir.ActivationFunctionType.Identity,
                                 bias=sm[0:1, _C_BRO:_C_BRO + 1], scale=1.0)
        nc.sync.dma_start(out[:, :], pred)

    nc.compile()
    return nc, n_cores


_CACHE = {}


def _get_built(mode: str):
    if mode not in _CACHE:
        _CACHE[mode] = _build(mode)
    return _CACHE[mode]


def _prep_inputs(mode, x, pos_emb, Wq, Wk, Wv, ln1_g, ln1_b, W1, b1, W2, b2,
                 ln2_g, ln2_b, Wro, bro):
    """Host-side shard + fold. Returns list of per-core input dicts."""
    n_cores = 8 if mode == "m8" else 4
    TL = T // 2 if mode == "m8" else T

    f32 = np.float32
    x = np.asarray(x).astype(np.int32)
    pos_emb = np.asarray(pos_emb, f32)
    Wq, Wk, Wv = (np.asarray(a, f32) for a in (Wq, Wk, Wv))
    ln1_g, ln1_b = np.asarray(ln1_g, f32), np.asarray(ln1_b, f32)
    W1, b1 = np.asarray(W1, f32), np.asarray(b1, f32)
    W2, b2 = np.asarray(W2, f32), np.asarray(b2, f32)
    ln2_g, ln2_b = np.asarray(ln2_g, f32), np.asarray(ln2_b, f32)
    Wro, bro = np.asarray(Wro, f32), np.asarray(bro, f32)

    # fold LN gains/biases into projections
    wqkT = np.stack([
        np.stack([(Wq[l] * ln1_g[l][None, :]).T,
                  (Wk[l] * ln1_g[l][None, :]).T]) for l in range(L)])
    wvT = np.stack([(Wv[l] * ln1_g[l][None, :]).T for l in range(L)])
    w1T = np.stack([(W1[l] * ln2_g[l][None, :]).T for l in range(L)])
    w2T = np.stack([W2[l].T for l in range(L)])

    smalls = np.zeros((D, 128), f32)
    for l in range(L):
        smalls[:, _C_CQKV + 3 * l + 0] = ln1_b[l] @ Wq[l].T
        smalls[:, _C_CQKV + 3 * l + 1] = ln1_b[l] @ Wk[l].T
        smalls[:, _C_CQKV + 3 * l + 2] = ln1_b[l] @ Wv[l].T
        c1 = (b1[l] + ln2_b[l] @ W1[l].T).reshape(HM // D, D)
        for m in range(HM // D):
            smalls[:, _C_C1 + 4 * l + m] = c1[m]
        smalls[:, _C_C2 + l] = b2[l]
    smalls[:, _C_WRO] = Wro[0]
    smalls[:, _C_BRO] = float(bro.reshape(-1)[0])

    ar = np.arange(D, dtype=f32)
    mask = (ar < V).astype(f32)
    aconst = np.stack([-0.5 * ar * ar * mask, ar * mask, -0.5 * mask])

    bf16 = ml_dtypes.bfloat16
    common = dict(
        aconst=np.ascontiguousarray(aconst, f32),
        wqkT=np.ascontiguousarray(wqkT.astype(bf16)),
        wvT=np.ascontiguousarray(wvT.astype(bf16)),
        w1T=np.ascontiguousarray(w1T.astype(bf16)),
        w2T=np.ascontiguousarray(w2T.astype(bf16)),
        smalls=np.ascontiguousarray(smalls, f32),
    )
    in_maps = []
    for c in range(n_cores):
        if mode == "m8":
            b_idx, half = c // 2, c % 2
            sl = slice(half * TL, (half + 1) * TL)
        else:
            b_idx, sl = c, slice(0, T)
        xrow = np.zeros((1, TL + 128), np.int32)
        xrow[0, :TL] = x[b_idx, sl]
        xrow[0, TL] = c % 2 if mode == "m8" else 0
        in_maps.append(dict(
            common,
            xin=xrow,
            pose=np.ascontiguousarray(pos_emb[sl]),
        ))
    return in_maps


def kernel(**inputs):
    mode = MODE
    nc, n_cores = _get_built(mode)
    in_maps = _prep_inputs(mode, **inputs)
    res = run_bass_kernel_spmd(nc, in_maps, core_ids=list(range(n_cores)))
    TL = T // 2 if mode == "m8" else T
    out = np.empty((B, T), np.float32)
    for c in range(n_cores):
        if mode == "m8":
            b_idx, half = c // 2, c % 2
            out[b_idx, half * TL:(half + 1) * TL] = res.results[c]["out"][0]
        else:
            out[c] = res.results[c]["out"][0]
    return out

